# revision 1
# baseline (speedup 1.0000x reference)
"""Fused cross-entropy label-propagation kernel for Trainium2 (8 cores).

Computation (per batch b):
  sim   = ref_flat(b) @ tgt_flat(b)          # [12288, 4096]
  prob  = softmax(sim, axis=0)               # over ref pixels
  pred  = lab_flat(b) @ prob                 # [16, 4096]
  loss  = mean(-log(pred[label] + eps))

Sharding: batch b = core // 4, target-pixel columns split 4-way per batch
(softmax is over the ref axis, so column sharding needs no communication).

Device kernel (per core, T_LOC = 1024 columns):
  for each of 96 ref-row tiles (128 rows):
    sim_psum[128, 1024]  = refT_tile.T @ tgt     (fp8e4m3 DoubleRow, K=256
                                                  in one pass, 0.5 cyc/row)
    p[128, 1024]         = exp(sim_psum - shift) (ScalarE, PSUM->SBUF bf16)
    pred_psum[17, 1024] += [lab|ones]_tile.T @ p (bf16, accumulate over tiles)
  out = pred_psum   # rows 0..15 = unnormalized label sums, row 16 = denom

fp8e4m3 for ref/target adds ~0.4 absolute noise to sim (rel loss error
~1.6e-3, measured against the fp32 reference -- well inside the 2e-2 gate)
and halves the PE's sim-matmul time; bf16 for p/labels keeps the exp window
(bf16 exponent range == fp32) while allowing 2-byte moving operands.

The constant shift replaces the per-column max. The data contains
near-collinear ref/target pairs, so per-column sim maxima span roughly
[57, 221] -- that fits float32/bf16's ~175-unit representable exp window
when centered with shift ~138.5. The shift is baked into the program (exp
bias); the host validates every column of the result (denominator finite
and positive) and, if any column falls outside the window, reruns with
the shift moved +-60 (a fresh compile, never hit on reference-like data)
and patches those columns. The shift cancels exactly in num/den.

Host finishes with num/den, log, gather, mean over 8192 elements (float64).
"""

import os

import numpy as np
import ml_dtypes

LAG = int(os.environ.get("KLAG", "3"))  # label-matmul lag behind exp, tiles

B, NREF, F, H, W, D = 2, 3, 256, 64, 64, 16
T = H * W                     # 4096 target pixels per batch
N = NREF * T                  # 12288 ref pixels per batch
NCORES = 8
T_LOC = B * T // NCORES       # 1024 columns per core
NT = N // 128                 # 96 ref-row tiles
NCHUNK = 8                    # ref DMA chunks (12 k-tiles each)
KPC = NT // NCHUNK            # k-tiles per chunk
SHIFT0 = 138.5                # subtracted from sim before exp (host-adjustable)
EPS = 1e-14

FP8 = ml_dtypes.float8_e4m3
BF16 = ml_dtypes.bfloat16

_CACHE = {}
LAST_RESULTS = None  # BassKernelResults of the most recent run (for profiling)


def _build_program(reps=1, shift=SHIFT0):
    # reps > 1 repeats the whole compute body (timing harness only; the extra
    # reps recompute the same result into the same output). The softmax shift
    # is baked in as the exp activation's bias immediate; rescue reruns with a
    # moved shift compile a fresh program (never triggered by reference-like
    # data, so the compile cost stays off the common path).
    if ("nc", reps, shift) in _CACHE:
        return _CACHE[("nc", reps, shift)]

    import concourse.bacc as bacc
    import concourse.tile as tile
    import concourse.mybir as mybir

    f32 = mybir.dt.float32
    bf16 = mybir.dt.bfloat16
    fp8 = mybir.dt.float8e4

    nc = bacc.Bacc("TRN2", target_bir_lowering=False, debug=False,
                   num_devices=NCORES)

    # Per-core inputs, pre-laid-out on host so every DMA is contiguous.
    ref_d = nc.dram_tensor("ref", [NCHUNK, 128, KPC, 2, 128], fp8,
                           kind="ExternalInput")
    tgt_d = nc.dram_tensor("tgt", [128, 2, T_LOC], fp8, kind="ExternalInput")
    lab_d = nc.dram_tensor("lab", [128, NT, D + 1], bf16,
                           kind="ExternalInput")
    out_d = nc.dram_tensor("out", [D + 1, T_LOC], f32, kind="ExternalOutput")

    with tile.TileContext(nc) as tc:
        with (
            tc.tile_pool(name="small", bufs=1) as small,
            tc.tile_pool(name="ppool", bufs=LAG + 2) as ppool,
            tc.tile_pool(name="simpool", bufs=3, space="PSUM") as simpool,
            tc.tile_pool(name="predpool", bufs=1, space="PSUM") as predpool,
        ):
            # Warm the ScalarE exp table immediately (the ~1.3us
            # ACT_TABLE_LOAD runs under the input DMAs instead of on the
            # critical path of the first real exp). The dummy reads the
            # uninitialized output staging tile (result is never consumed),
            # so no input DMA or memset gates it.
            po0 = small.tile([D + 1, 512], f32, tag="po0")
            po1 = small.tile([D + 1, 512], f32, tag="po1")
            dummy = small.tile([D + 1, 1], f32, tag="dummy")
            nc.scalar.activation(out=dummy, in_=po0[:, 0:1],
                                 func=mybir.ActivationFunctionType.Exp,
                                 scale=1.0)

            # Startup-critical loads split across issue queues so the first
            # sim matmul + exp only wait on tiny transfers: first tgt half on
            # SP; first two ref k-tiles + first lab rows via GpSimd SWDGE.
            tgt_sb = small.tile([128, 2, T_LOC], fp8, tag="tgt")
            nc.sync.dma_start(out=tgt_sb[:, :, 0:512], in_=tgt_d[:, :, 0:512])
            ref_sb = small.tile([128, NT, 2, 128], fp8, tag="ref")
            nc.gpsimd.dma_start(out=ref_sb[:, 0:2], in_=ref_d[0][:, 0:2])
            bias_sb = small.tile([128, 1], f32, tag="bias")
            nc.gpsimd.memset(bias_sb, -shift)
            lab_sb = small.tile([128, NT, D + 1], bf16, tag="lab")
            nc.gpsimd.dma_start(out=lab_sb[:, 0:8], in_=lab_d[:, 0:8])
            nc.sync.dma_start(out=tgt_sb[:, :, 512:], in_=tgt_d[:, :, 512:])
            nc.sync.dma_start(out=ref_sb[:, 2:5], in_=ref_d[0][:, 2:5])
            nc.sync.dma_start(out=ref_sb[:, 5:KPC], in_=ref_d[0][:, 5:KPC])
            nc.sync.dma_start(out=lab_sb[:, 8:], in_=lab_d[:, 8:])
            for c in range(1, NCHUNK):
                nc.sync.dma_start(out=ref_sb[:, c * KPC:(c + 1) * KPC],
                                  in_=ref_d[c])

            def label_mm(k, p, pred):
                for cc in range(2):
                    nc.tensor.matmul(
                        pred[:, cc * 512:(cc + 1) * 512],
                        lhsT=lab_sb[:, k],
                        rhs=p[:, cc * 512:(cc + 1) * 512],
                        start=(k == 0), stop=(k == NT - 1),
                    )

            drain_count = [0]

            def drain(pred):
                # Output drain stays off the ScalarE queue so the next rep's
                # exps aren't stuck behind it at the rep boundary. On the
                # final rep ScalarE has no more exps to run, so its copy of
                # the second half overlaps the DVE copy of the first.
                drain_count[0] += 1
                final = drain_count[0] == reps
                if final:
                    nc.scalar.copy(po1, pred[:, 512:])
                    nc.gpsimd.dma_start(out=out_d[:, 512:], in_=po1)
                    nc.vector.tensor_copy(po0, pred[:, :512])
                    nc.sync.dma_start(out=out_d[:, :512], in_=po0)
                else:
                    nc.vector.tensor_copy(po0, pred[:, :512])
                    nc.sync.dma_start(out=out_d[:, :512], in_=po0)
                    nc.vector.tensor_copy(po1, pred[:, 512:])
                    nc.gpsimd.dma_start(out=out_d[:, 512:], in_=po1)

            # (k, p, pred) label matmuls lagged LAG tiles so the PE never
            # waits on the exp of the preceding tiles (HW A/B: LAG>=2 is a
            # few us/rep faster than LAG=1). The queue carries across rep
            # boundaries so the next rep's sims keep ScalarE fed while the
            # previous rep's tail labels and drain retire.
            pending = []
            for rep in range(reps):
                pred = predpool.tile([D + 1, T_LOC], f32, tag="pred")
                for k in range(NT):
                    sim = simpool.tile([128, T_LOC], f32, tag="sim")
                    for cc in range(2):
                        nc.tensor.matmul(
                            sim[:, cc * 512:(cc + 1) * 512],
                            lhsT=ref_sb[:, k],
                            rhs=tgt_sb[:, :, cc * 512:(cc + 1) * 512],
                            start=True, stop=True,
                            perf_mode=mybir.MatmulPerfMode.DoubleRow,
                        )
                    p = ppool.tile([128, T_LOC], bf16, tag="p")
                    nc.scalar.activation(out=p, in_=sim,
                                         func=mybir.ActivationFunctionType.Exp,
                                         bias=bias_sb[:], scale=1.0)
                    pending.append((k, p, pred))
                    if len(pending) > LAG:
                        ent = pending.pop(0)
                        label_mm(*ent)
                        if ent[0] == NT - 1:
                            drain(ent[2])
            while pending:
                ent = pending.pop(0)
                label_mm(*ent)
                if ent[0] == NT - 1:
                    drain(ent[2])

    nc.compile()
    _CACHE[("nc", reps, shift)] = nc
    return nc


def _prep_inputs(ref, target, ref_label):
    """Per-batch host-side relayouts shared by the 4 cores of each batch."""
    per_b = []
    for b in range(B):
        # ref tile layout for DoubleRow: [chunk, f_lo(part), k_in_chunk,
        # j(f_hi), n_in_tile], fp8e4m3
        rf = ref[b].astype(FP8)                      # [3, 256, 64, 64]
        rf = rf.reshape(NREF, 2, 128, T)             # [r, j, f_lo, hw]
        rf = rf.transpose(0, 3, 1, 2)                # [r, hw, j, f_lo]
        rf = rf.reshape(NT, 128, 2, 128)             # [k, nn, j, f_lo]
        rf = rf.transpose(0, 3, 2, 1)                # [k, f_lo, j, nn]
        rf = rf.reshape(NCHUNK, KPC, 128, 2, 128)
        refb = np.ascontiguousarray(rf.transpose(0, 2, 1, 3, 4))
        # target: [f_lo(part), j, t], fp8
        tg = target[b].astype(FP8).reshape(2, 128, T)
        tgtb = np.ascontiguousarray(tg.transpose(1, 0, 2))
        # labels: n = (r, h, w) major -> [12288, 16], append ones -> [.., 17]
        labn = ref_label[b].transpose(0, 2, 3, 1).reshape(N, D)
        labo = np.concatenate(
            [labn, np.ones((N, 1), np.float32)], axis=1)
        # -> SBUF layout [128(part), 96, 17]: sb[p, k, j] = labo[k*128+p, j]
        labsb = np.ascontiguousarray(
            labo.reshape(NT, 128, D + 1).transpose(1, 0, 2)).astype(BF16)
        per_b.append((refb, labsb, tgtb))
    return per_b


def _run_cores(per_b, shift):
    """One SPMD run with the given softmax shift; returns per-batch [17, 4096]."""
    global LAST_RESULTS
    from concourse.bass_utils import run_bass_kernel_spmd

    nc = _build_program(shift=shift)
    in_maps = []
    for core in range(NCORES):
        b, s = divmod(core, NCORES // B)
        refb, labsb, tgtb = per_b[b]
        in_maps.append({
            "ref": refb,
            "tgt": np.ascontiguousarray(tgtb[:, :, s * T_LOC:(s + 1) * T_LOC]),
            "lab": labsb,
        })
    LAST_RESULTS = run_bass_kernel_spmd(nc, in_maps, list(range(NCORES)))
    outs = LAST_RESULTS.results
    return [
        np.concatenate(
            [outs[b * (NCORES // B) + s]["out"] for s in range(NCORES // B)],
            axis=1).astype(np.float64)
        for b in range(B)
    ]


def _bad_cols(raw):
    """Columns whose exp window overflowed/underflowed for the used shift."""
    with np.errstate(all="ignore"):
        den, num = raw[D], raw[:D]
        return ~np.isfinite(den) | (den <= 0.0) | ~np.isfinite(num).all(axis=0)


def kernel(ref, target, ref_label, target_label):
    ref = np.asarray(ref, np.float32)
    target = np.asarray(target, np.float32)
    ref_label = np.asarray(ref_label, np.float32)
    labels = np.asarray(target_label).astype(np.int64)

    per_b = _prep_inputs(ref, target, ref_label)
    raws = _run_cores(per_b, SHIFT0)

    # Rescue any columns outside the exp window with shifted reruns (a no-op
    # for data resembling the reference distribution).
    bad = [_bad_cols(r) for r in raws]
    for delta in (60.0, -60.0, 120.0, -120.0):
        if not any(bm.any() for bm in bad):
            break
        raws2 = _run_cores(per_b, SHIFT0 + delta)
        for b in range(B):
            fixable = bad[b] & ~_bad_cols(raws2[b])
            raws[b][:, fixable] = raws2[b][:, fixable]
            bad[b] &= ~fixable

    nll_sum = 0.0
    with np.errstate(all="ignore"):
        for b in range(B):
            pred = raws[b][:D] / raws[b][D]                  # [16, 4096]
            logp = np.log(pred + EPS)
            idx = labels[b].reshape(T)
            nll_sum += -logp[idx, np.arange(T)].sum()
    loss = nll_sum / (B * T)
    return np.asarray(loss, dtype=np.float32)



# revision 4
# speedup vs baseline: 1.2311x; 1.2311x over previous
"""Fused cross-entropy label-propagation kernel for Trainium2 (8 cores), v2.

Computation (per batch b):
  sim   = ref_flat(b) @ tgt_flat(b)          # [12288, 4096]
  prob  = softmax(sim, axis=0)               # over ref pixels
  pred  = lab_flat(b) @ prob                 # [16, 4096]
  loss  = mean(-log(pred[label] + eps))

Sharding: batch b = core // 4, target-pixel columns split 4-way per batch
(softmax is over the ref axis, so column sharding needs no communication).

v2 changes over the ACT-bound v1 (98.6us):
1. The exp of all 12.6M sim values per core was the bottleneck (ScalarE is
   the only exp engine, 1 elem/cycle/lane @1.2GHz = 82us floor). The exp
   is now SPLIT between ScalarE (exact exp) and the DVE using Schraudolph's
   trick: host prescales ref/tgt by sqrt(128*log2 e) so the PE produces
   sim' = 128*log2e*s in PSUM; the DVE then computes
   int16(max(sim' + B_ADD, 0)) in one tensor_scalar op, whose int16 bit
   pattern reinterpreted as bf16 equals e^(s-shift) within +-3%. The
   pipeline already tolerates +-49% per-element noise from fp8 (measured
   1.6e-3 final loss error), so +-3% on half the tiles is noise
   (measured numerically: 4e-4 final rel error).
2. The label matmul (M=17, 13% PE array use) is packed 4x with col-tiling:
   4 k-tiles' matmuls run concurrently in four 32-column groups of the PE
   array, accumulating into four partition-slices of a [128, 1024] PSUM
   tile. The host sums the 4 slices. 41us -> ~11us of PE time.
3. Exp instructions are 1536 wide (3 PSUM banks, 1.5 ref-row tiles) to
   amortize the fixed ~185ns SBUF-access cost per ACT instruction.

PSUM: 2 sim slots (3 banks each) + pred (2 banks) = 8 banks exactly.

The constant shift replaces the per-column max (data-dependent rescue on
the host reruns with shift +-60/120 if any column's exp window overflowed;
never triggered on reference-like data, where col maxima are 57..220).
Schraudolph saturation at s-shift > 97.6 would produce NaN/garbage in the
affected column; the same host check catches that case too.

Host finishes with slice-combine, num/den, log, gather, mean (float64).
"""

import math
import os

import numpy as np
import ml_dtypes

B, NREF, F, H, W, D = 2, 3, 256, 64, 64, 16
T = H * W                     # 4096 target pixels per batch
N = NREF * T                  # 12288 ref pixels per batch
NCORES = 8
T_LOC = B * T // NCORES       # 1024 columns per core
NT = N // 128                 # 96 ref-row tiles
NU = 2 * NT                   # 192 column-units of 512 per core
SLOT = 1536                   # exp slot width (3 PSUM banks)
NSLOT = NU * 512 // SLOT      # 64 exp slots
NPACK = NT // 4               # 24 col-tiled label packs
NCHUNK = 8                    # ref DMA chunks (12 k-tiles each)
KPC = NT // NCHUNK            # k-tiles per chunk
SHIFT0 = 138.5                # subtracted from sim before exp (host-adjustable)
EPS = 1e-14
LOG2E = math.log2(math.e)
A_SCALE = 128.0 * LOG2E       # PE computes sim' = A_SCALE * s
SQA = math.sqrt(A_SCALE)      # host folds sqrt into each fp8 operand
SIGMA = 5.5104                # Schraudolph bias: min-max relative error

LAG = int(os.environ.get("KLAG", "2"))          # label packs lag, in slots
ACT_SHARE = float(os.environ.get("KACT", "0.541"))  # exp share on ScalarE
PBUFS = int(os.environ.get("KPBUFS", str(LAG + 5)))

FP8 = ml_dtypes.float8_e4m3
BF16 = ml_dtypes.bfloat16

_CACHE = {}
LAST_RESULTS = None  # BassKernelResults of the most recent run (for profiling)


def _exp_schedule():
    """Per-slot engine assignment: 'A' (ScalarE exact exp) or 'V' (DVE
    Schraudolph). Interleaved to keep both engines fed."""
    sched = []
    cum = 0.0
    for _ in range(NSLOT):
        cum += ACT_SHARE
        if cum >= 1.0:
            sched.append("A")
            cum -= 1.0
        else:
            sched.append("V")
    return sched


def _build_program(reps=1, shift=SHIFT0):
    # reps > 1 repeats the whole compute body (timing harness only; the extra
    # reps recompute the same result into the same output).
    key = ("nc", reps, shift, LAG, ACT_SHARE, PBUFS)
    if key in _CACHE:
        return _CACHE[key]

    import concourse.bacc as bacc
    import concourse.tile as tile
    import concourse.mybir as mybir

    f32 = mybir.dt.float32
    bf16 = mybir.dt.bfloat16
    i16 = mybir.dt.int16
    fp8 = mybir.dt.float8e4

    b_add = 128.0 * (127.0 - LOG2E * shift) - SIGMA
    sched = _exp_schedule()

    nc = bacc.Bacc("TRN2", target_bir_lowering=False, debug=False,
                   num_devices=NCORES)

    # Per-core inputs, pre-laid-out on host so every DMA is contiguous.
    ref_d = nc.dram_tensor("ref", [NCHUNK, 128, KPC, 2, 128], fp8,
                           kind="ExternalInput")
    tgt_d = nc.dram_tensor("tgt", [128, 2, T_LOC], fp8, kind="ExternalInput")
    lab_d = nc.dram_tensor("lab", [128, NT, D + 1], bf16,
                           kind="ExternalInput")
    out_d = nc.dram_tensor("out", [128, T_LOC], f32, kind="ExternalOutput")

    with tile.TileContext(nc) as tc:
        with (
            tc.tile_pool(name="small", bufs=1) as small,
            tc.tile_pool(name="ppool", bufs=PBUFS) as ppool,
            tc.tile_pool(name="simpool", bufs=2, space="PSUM") as simpool,
            tc.tile_pool(name="predpool", bufs=1, space="PSUM") as predpool,
        ):
            # Warm the ScalarE exp table immediately (the ~2.7us
            # ACT_TABLE_LOAD runs under the input DMAs instead of on the
            # critical path of the first real exp).
            po0 = small.tile([128, 512], f32, tag="po0")
            po1 = small.tile([128, 512], f32, tag="po1")
            dummy = small.tile([128, 1], f32, tag="dummy")
            nc.scalar.activation(out=dummy, in_=po0[:, 0:1],
                                 func=mybir.ActivationFunctionType.Exp,
                                 scale=1.0)

            # Startup-critical loads split across issue queues so the first
            # sim matmul + exp only wait on tiny transfers.
            tgt_sb = small.tile([128, 2, T_LOC], fp8, tag="tgt")
            nc.sync.dma_start(out=tgt_sb[:, :, 0:512], in_=tgt_d[:, :, 0:512])
            ref_sb = small.tile([128, NT, 2, 128], fp8, tag="ref")
            nc.gpsimd.dma_start(out=ref_sb[:, 0:2], in_=ref_d[0][:, 0:2])
            bias_sb = small.tile([128, 1], f32, tag="bias")
            nc.gpsimd.memset(bias_sb, -shift)
            lab_sb = small.tile([128, NT, D + 1], bf16, tag="lab")
            nc.gpsimd.dma_start(out=lab_sb[:, 0:8], in_=lab_d[:, 0:8])
            nc.sync.dma_start(out=tgt_sb[:, :, 512:], in_=tgt_d[:, :, 512:])
            nc.sync.dma_start(out=ref_sb[:, 2:5], in_=ref_d[0][:, 2:5])
            nc.sync.dma_start(out=ref_sb[:, 5:KPC], in_=ref_d[0][:, 5:KPC])
            nc.sync.dma_start(out=lab_sb[:, 8:], in_=lab_d[:, 8:])
            for c in range(1, NCHUNK):
                nc.sync.dma_start(out=ref_sb[:, c * KPC:(c + 1) * KPC],
                                  in_=ref_d[c])

            # Label pack q covers k-tiles 4q..4q+3; its p units live in
            # slots up to (8q+7)//3. Emitted LAG slots after that.
            ready_slot = [(8 * q + 7) // 3 for q in range(NPACK)]

            def label_pack(q, slot_p, pred):
                for h in range(2):
                    for j in range(4):
                        k = 4 * q + j
                        u = 2 * k + h
                        sl, off = divmod(u * 512, SLOT)
                        nc.tensor.matmul(
                            pred[32 * j:32 * j + 17, h * 512:(h + 1) * 512],
                            lhsT=lab_sb[:, k],
                            rhs=slot_p[sl][:, off:off + 512],
                            start=(q == 0), stop=(q == NPACK - 1),
                            tile_position=(0, 32 * j),
                        )

            def drain(pred):
                nc.vector.tensor_copy(po0, pred[:, :512])
                nc.sync.dma_start(out=out_d[:, :512], in_=po0)
                nc.scalar.copy(po1, pred[:, 512:])
                nc.gpsimd.dma_start(out=out_d[:, 512:], in_=po1)

            for rep in range(reps):
                pred = predpool.tile([128, T_LOC], f32, tag="pred")
                slot_p = {}
                nextq = 0
                for s in range(NSLOT):
                    sim = simpool.tile([128, SLOT], f32, tag="sim")
                    for i in range(3):
                        u = 3 * s + i
                        k, h = divmod(u, 2)
                        nc.tensor.matmul(
                            sim[:, 512 * i:512 * (i + 1)],
                            lhsT=ref_sb[:, k],
                            rhs=tgt_sb[:, :, 512 * h:512 * (h + 1)],
                            start=True, stop=True,
                            perf_mode=mybir.MatmulPerfMode.DoubleRow,
                        )
                    p = ppool.tile([128, SLOT], bf16, tag="p")
                    if sched[s] == "A":
                        nc.scalar.activation(
                            out=p, in_=sim,
                            func=mybir.ActivationFunctionType.Exp,
                            bias=bias_sb[:], scale=1.0 / A_SCALE)
                    else:
                        nc.vector.tensor_scalar(
                            out=p.bitcast(i16), in0=sim,
                            scalar1=b_add, scalar2=0.0,
                            op0=mybir.AluOpType.add,
                            op1=mybir.AluOpType.max)
                    slot_p[s] = p
                    while nextq < NPACK and ready_slot[nextq] + LAG <= s:
                        label_pack(nextq, slot_p, pred)
                        nextq += 1
                while nextq < NPACK:
                    label_pack(nextq, slot_p, pred)
                    nextq += 1
                drain(pred)

    nc.compile()
    _CACHE[key] = nc
    return nc


def _prep_inputs(ref, target, ref_label):
    """Per-batch host-side relayouts shared by the 4 cores of each batch.
    ref/target are prescaled by sqrt(128*log2 e) so the PE's sim output is
    already in Schraudolph exponent units."""
    per_b = []
    for b in range(B):
        # ref tile layout for DoubleRow: [chunk, f_lo(part), k_in_chunk,
        # j(f_hi), n_in_tile], fp8e4m3
        rf = (ref[b] * SQA).astype(FP8)              # [3, 256, 64, 64]
        rf = rf.reshape(NREF, 2, 128, T)             # [r, j, f_lo, hw]
        rf = rf.transpose(0, 3, 1, 2)                # [r, hw, j, f_lo]
        rf = rf.reshape(NT, 128, 2, 128)             # [k, nn, j, f_lo]
        rf = rf.transpose(0, 3, 2, 1)                # [k, f_lo, j, nn]
        rf = rf.reshape(NCHUNK, KPC, 128, 2, 128)
        refb = np.ascontiguousarray(rf.transpose(0, 2, 1, 3, 4))
        # target: [f_lo(part), j, t], fp8
        tg = (target[b] * SQA).astype(FP8).reshape(2, 128, T)
        tgtb = np.ascontiguousarray(tg.transpose(1, 0, 2))
        # labels: n = (r, h, w) major -> [12288, 16], append ones -> [.., 17]
        labn = ref_label[b].transpose(0, 2, 3, 1).reshape(N, D)
        labo = np.concatenate(
            [labn, np.ones((N, 1), np.float32)], axis=1)
        # -> SBUF layout [128(part), 96, 17]: sb[p, k, j] = labo[k*128+p, j]
        labsb = np.ascontiguousarray(
            labo.reshape(NT, 128, D + 1).transpose(1, 0, 2)).astype(BF16)
        per_b.append((refb, labsb, tgtb))
    return per_b


def _run_cores(per_b, shift):
    """One SPMD run with the given softmax shift; returns per-batch
    [17, 4096] float64 (the four col-tiled partition slices summed)."""
    global LAST_RESULTS
    from concourse.bass_utils import run_bass_kernel_spmd

    nc = _build_program(shift=shift)
    in_maps = []
    for core in range(NCORES):
        b, s = divmod(core, NCORES // B)
        refb, labsb, tgtb = per_b[b]
        in_maps.append({
            "ref": refb,
            "tgt": np.ascontiguousarray(tgtb[:, :, s * T_LOC:(s + 1) * T_LOC]),
            "lab": labsb,
        })
    LAST_RESULTS = run_bass_kernel_spmd(nc, in_maps, list(range(NCORES)))
    outs = LAST_RESULTS.results
    res = []
    for b in range(B):
        cols = []
        for s in range(NCORES // B):
            raw = outs[b * (NCORES // B) + s]["out"].astype(np.float64)
            comb = sum(raw[32 * j:32 * j + D + 1] for j in range(4))
            cols.append(comb)
        res.append(np.concatenate(cols, axis=1))
    return res


def _bad_cols(raw):
    """Columns whose exp window overflowed/underflowed for the used shift."""
    with np.errstate(all="ignore"):
        den, num = raw[D], raw[:D]
        return ~np.isfinite(den) | (den <= 0.0) | ~np.isfinite(num).all(axis=0)


def kernel(ref, target, ref_label, target_label):
    ref = np.asarray(ref, np.float32)
    target = np.asarray(target, np.float32)
    ref_label = np.asarray(ref_label, np.float32)
    labels = np.asarray(target_label).astype(np.int64)

    per_b = _prep_inputs(ref, target, ref_label)
    raws = _run_cores(per_b, SHIFT0)

    # Rescue any columns outside the exp window with shifted reruns (a no-op
    # for data resembling the reference distribution).
    bad = [_bad_cols(r) for r in raws]
    for delta in (60.0, -60.0, 120.0, -120.0):
        if not any(bm.any() for bm in bad):
            break
        raws2 = _run_cores(per_b, SHIFT0 + delta)
        for b in range(B):
            fixable = bad[b] & ~_bad_cols(raws2[b])
            raws[b][:, fixable] = raws2[b][:, fixable]
            bad[b] &= ~fixable

    nll_sum = 0.0
    with np.errstate(all="ignore"):
        for b in range(B):
            pred = raws[b][:D] / raws[b][D]                  # [16, 4096]
            logp = np.log(pred + EPS)
            idx = labels[b].reshape(T)
            nll_sum += -logp[idx, np.arange(T)].sum()
    loss = nll_sum / (B * T)
    return np.asarray(loss, dtype=np.float32)


# revision 9
# speedup vs baseline: 1.4793x; 1.2016x over previous
"""Fused cross-entropy label-propagation kernel for Trainium2 (8 cores), v2.

Computation (per batch b):
  sim   = ref_flat(b) @ tgt_flat(b)          # [12288, 4096]
  prob  = softmax(sim, axis=0)               # over ref pixels
  pred  = lab_flat(b) @ prob                 # [16, 4096]
  loss  = mean(-log(pred[label] + eps))

Sharding: batch b = core // 4, target-pixel columns split 4-way per batch
(softmax is over the ref axis, so column sharding needs no communication).

v2 changes over the ACT-bound v1 (98.6us):
1. The exp of all 12.6M sim values per core was the bottleneck (ScalarE is
   the only exp engine, 1 elem/cycle/lane @1.2GHz = 82us floor). The exp
   is now SPLIT between ScalarE (exact exp) and the DVE using Schraudolph's
   trick: host prescales ref/tgt by sqrt(128*log2 e) so the PE produces
   sim' = 128*log2e*s in PSUM; the DVE then computes
   int16(max(sim' + B_ADD, 0)) in one tensor_scalar op, whose int16 bit
   pattern reinterpreted as bf16 equals e^(s-shift) within +-3%. The
   pipeline already tolerates +-49% per-element noise from fp8 (measured
   1.6e-3 final loss error), so +-3% on half the tiles is noise
   (measured numerically: 4e-4 final rel error).
2. The label matmul (M=17, 13% PE array use) is packed 4x with col-tiling:
   4 k-tiles' matmuls run concurrently in four 32-column groups of the PE
   array, accumulating into four partition-slices of a [128, 1024] PSUM
   tile. The host sums the 4 slices. 41us -> ~11us of PE time.
3. Exp instructions are 1536 wide (3 PSUM banks, 1.5 ref-row tiles) to
   amortize the fixed ~185ns SBUF-access cost per ACT instruction.

PSUM: 2 sim slots (3 banks each) + pred (2 banks) = 8 banks exactly.

The constant shift replaces the per-column max (data-dependent rescue on
the host reruns with shift +-60/120 if any column's exp window overflowed;
never triggered on reference-like data, where col maxima are 57..220).
Schraudolph saturation at s-shift > 97.6 would produce NaN/garbage in the
affected column; the same host check catches that case too.

Host finishes with slice-combine, num/den, log, gather, mean (float64).
"""

import math
import os

import numpy as np
import ml_dtypes

B, NREF, F, H, W, D = 2, 3, 256, 64, 64, 16
T = H * W                     # 4096 target pixels per batch
N = NREF * T                  # 12288 ref pixels per batch
NCORES = 8
T_LOC = B * T // NCORES       # 1024 columns per core
NT = N // 128                 # 96 ref-row tiles (one exp slot each)
NSLOT = NT                    # exp slots: [128, 1024], 2 PSUM banks
NPACK = NT // 4               # 24 col-tiled label packs
NCHUNK = 8                    # ref DMA chunks (12 k-tiles each)
KPC = NT // NCHUNK            # k-tiles per chunk
SHIFT0 = 138.5                # subtracted from sim before exp (host-adjustable)
EPS = 1e-14
LOG2E = math.log2(math.e)
A_SCALE = 128.0 * LOG2E       # PE computes sim' = A_SCALE * s
SQA = math.sqrt(A_SCALE)      # host folds sqrt into each fp8 operand
SIGMA = 5.5104                # Schraudolph bias: min-max relative error

LAG = int(os.environ.get("KLAG", "2"))          # label packs lag, in slots
ACT_SHARE = float(os.environ.get("KACT", "0.535"))  # exp share on ScalarE
PBUFS = int(os.environ.get("KPBUFS", str(LAG + 7)))

FP8 = ml_dtypes.float8_e4m3
BF16 = ml_dtypes.bfloat16

_CACHE = {}
LAST_RESULTS = None  # BassKernelResults of the most recent run (for profiling)


def _exp_schedule():
    """Per-slot engine assignment: 'A' (ScalarE exact exp) or 'V' (DVE
    Schraudolph). Interleaved to keep both engines fed."""
    sched = []
    cum = 0.0
    for _ in range(NSLOT):
        cum += ACT_SHARE
        if cum >= 1.0:
            sched.append("A")
            cum -= 1.0
        else:
            sched.append("V")
    return sched


def _build_program(reps=1, shift=SHIFT0):
    # reps > 1 repeats the whole compute body (timing harness only; the extra
    # reps recompute the same result into the same output).
    key = ("nc", reps, shift, LAG, ACT_SHARE, PBUFS)
    if key in _CACHE:
        return _CACHE[key]

    import concourse.bacc as bacc
    import concourse.tile as tile
    import concourse.mybir as mybir

    f32 = mybir.dt.float32
    bf16 = mybir.dt.bfloat16
    i16 = mybir.dt.int16
    fp8 = mybir.dt.float8e4

    b_add = 128.0 * (127.0 - LOG2E * shift) - SIGMA
    sched = _exp_schedule()

    nc = bacc.Bacc("TRN2", target_bir_lowering=False, debug=False,
                   num_devices=NCORES)

    # Per-core inputs, pre-laid-out on host so every DMA is contiguous.
    ref_d = nc.dram_tensor("ref", [NCHUNK, 128, KPC, 2, 128], fp8,
                           kind="ExternalInput")
    tgt_d = nc.dram_tensor("tgt", [128, 2, T_LOC], fp8, kind="ExternalInput")
    lab_d = nc.dram_tensor("lab", [128, NT, D + 1], bf16,
                           kind="ExternalInput")
    out_d = nc.dram_tensor("out", [128, T_LOC], f32, kind="ExternalOutput")

    with tile.TileContext(nc) as tc:
        with (
            tc.tile_pool(name="small", bufs=1) as small,
            tc.tile_pool(name="ppool", bufs=PBUFS) as ppool,
            tc.tile_pool(name="simpool", bufs=3, space="PSUM") as simpool,
            tc.tile_pool(name="predpool", bufs=1, space="PSUM") as predpool,
        ):
            # Warm the ScalarE exp table immediately (the ~2.7us
            # ACT_TABLE_LOAD runs under the input DMAs instead of on the
            # critical path of the first real exp).
            po0 = small.tile([128, 512], f32, tag="po0")
            po1 = small.tile([128, 512], f32, tag="po1")
            dummy = small.tile([128, 1], f32, tag="dummy")
            nc.scalar.activation(out=dummy, in_=po0[:, 0:1],
                                 func=mybir.ActivationFunctionType.Exp,
                                 scale=1.0)

            # Startup-critical loads split across issue queues so the first
            # sim matmul + exp only wait on tiny transfers.
            tgt_sb = small.tile([128, 2, T_LOC], fp8, tag="tgt")
            nc.sync.dma_start(out=tgt_sb[:, :, 0:512], in_=tgt_d[:, :, 0:512])
            ref_sb = small.tile([128, NT, 2, 128], fp8, tag="ref")
            nc.gpsimd.dma_start(out=ref_sb[:, 0:2], in_=ref_d[0][:, 0:2])
            bias_sb = small.tile([128, 1], f32, tag="bias")
            nc.gpsimd.memset(bias_sb, -shift)
            lab_sb = small.tile([128, NT, D + 1], bf16, tag="lab")
            nc.gpsimd.dma_start(out=lab_sb[:, 0:8], in_=lab_d[:, 0:8])
            nc.sync.dma_start(out=tgt_sb[:, :, 512:], in_=tgt_d[:, :, 512:])
            nc.sync.dma_start(out=ref_sb[:, 2:5], in_=ref_d[0][:, 2:5])
            nc.sync.dma_start(out=ref_sb[:, 5:KPC], in_=ref_d[0][:, 5:KPC])
            nc.sync.dma_start(out=lab_sb[:, 8:], in_=lab_d[:, 8:])
            for c in range(1, NCHUNK):
                nc.sync.dma_start(out=ref_sb[:, c * KPC:(c + 1) * KPC],
                                  in_=ref_d[c])

            # Label pack q covers k-tiles 4q..4q+3 (p slots 4q..4q+3, ready
            # at slot 4q+3); emitted LAG slots after that. The four j's run
            # concurrently in distinct 32-column groups of the PE array.
            def label_pack(q, slot_p, pred):
                for h in range(2):
                    for j in range(4):
                        k = 4 * q + j
                        nc.tensor.matmul(
                            pred[32 * j:32 * j + 17, h * 512:(h + 1) * 512],
                            lhsT=lab_sb[:, k],
                            rhs=slot_p[k][:, h * 512:(h + 1) * 512],
                            start=(q == 0), stop=(q == NPACK - 1),
                            tile_position=(0, 32 * j),
                        )

            def drain(pred):
                nc.vector.tensor_copy(po0, pred[:, :512])
                nc.sync.dma_start(out=out_d[:, :512], in_=po0)
                nc.scalar.copy(po1, pred[:, 512:])
                nc.gpsimd.dma_start(out=out_d[:, 512:], in_=po1)

            for rep in range(reps):
                pred = predpool.tile([128, T_LOC], f32, tag="pred")
                slot_p = {}
                nextq = 0
                for k in range(NSLOT):
                    sim = simpool.tile([128, T_LOC], f32, tag="sim")
                    for h in range(2):
                        nc.tensor.matmul(
                            sim[:, 512 * h:512 * (h + 1)],
                            lhsT=ref_sb[:, k],
                            rhs=tgt_sb[:, :, 512 * h:512 * (h + 1)],
                            start=True, stop=True,
                            perf_mode=mybir.MatmulPerfMode.DoubleRow,
                        )
                    p = ppool.tile([128, T_LOC], bf16, tag="p")
                    if sched[k] == "A":
                        nc.scalar.activation(
                            out=p, in_=sim,
                            func=mybir.ActivationFunctionType.Exp,
                            bias=bias_sb[:], scale=1.0 / A_SCALE)
                    else:
                        nc.vector.tensor_scalar(
                            out=p.bitcast(i16), in0=sim,
                            scalar1=b_add, scalar2=0.0,
                            op0=mybir.AluOpType.add,
                            op1=mybir.AluOpType.max)
                    slot_p[k] = p
                    while nextq < NPACK and 4 * nextq + 3 + LAG <= k:
                        label_pack(nextq, slot_p, pred)
                        nextq += 1
                while nextq < NPACK:
                    label_pack(nextq, slot_p, pred)
                    nextq += 1
                drain(pred)

    nc.compile()
    _CACHE[key] = nc
    return nc


def _prep_inputs(ref, target, ref_label):
    """Per-batch host-side relayouts shared by the 4 cores of each batch.
    ref/target are prescaled by sqrt(128*log2 e) so the PE's sim output is
    already in Schraudolph exponent units."""
    per_b = []
    for b in range(B):
        # ref tile layout for DoubleRow: [chunk, f_lo(part), k_in_chunk,
        # j(f_hi), n_in_tile], fp8e4m3
        rf = (ref[b] * SQA).astype(FP8)              # [3, 256, 64, 64]
        rf = rf.reshape(NREF, 2, 128, T)             # [r, j, f_lo, hw]
        rf = rf.transpose(0, 3, 1, 2)                # [r, hw, j, f_lo]
        rf = rf.reshape(NT, 128, 2, 128)             # [k, nn, j, f_lo]
        rf = rf.transpose(0, 3, 2, 1)                # [k, f_lo, j, nn]
        rf = rf.reshape(NCHUNK, KPC, 128, 2, 128)
        refb = np.ascontiguousarray(rf.transpose(0, 2, 1, 3, 4))
        # target: [f_lo(part), j, t], fp8
        tg = (target[b] * SQA).astype(FP8).reshape(2, 128, T)
        tgtb = np.ascontiguousarray(tg.transpose(1, 0, 2))
        # labels: n = (r, h, w) major -> [12288, 16], append ones -> [.., 17]
        labn = ref_label[b].transpose(0, 2, 3, 1).reshape(N, D)
        labo = np.concatenate(
            [labn, np.ones((N, 1), np.float32)], axis=1)
        # -> SBUF layout [128(part), 96, 17]: sb[p, k, j] = labo[k*128+p, j]
        labsb = np.ascontiguousarray(
            labo.reshape(NT, 128, D + 1).transpose(1, 0, 2)).astype(BF16)
        per_b.append((refb, labsb, tgtb))
    return per_b


def _run_cores(per_b, shift):
    """One SPMD run with the given softmax shift; returns per-batch
    [17, 4096] float64 (the four col-tiled partition slices summed)."""
    global LAST_RESULTS
    from concourse.bass_utils import run_bass_kernel_spmd

    nc = _build_program(shift=shift)
    in_maps = []
    for core in range(NCORES):
        b, s = divmod(core, NCORES // B)
        refb, labsb, tgtb = per_b[b]
        in_maps.append({
            "ref": refb,
            "tgt": np.ascontiguousarray(tgtb[:, :, s * T_LOC:(s + 1) * T_LOC]),
            "lab": labsb,
        })
    LAST_RESULTS = run_bass_kernel_spmd(nc, in_maps, list(range(NCORES)))
    outs = LAST_RESULTS.results
    res = []
    for b in range(B):
        cols = []
        for s in range(NCORES // B):
            raw = outs[b * (NCORES // B) + s]["out"].astype(np.float64)
            comb = sum(raw[32 * j:32 * j + D + 1] for j in range(4))
            cols.append(comb)
        res.append(np.concatenate(cols, axis=1))
    return res


def _bad_cols(raw):
    """Columns whose exp window overflowed/underflowed for the used shift."""
    with np.errstate(all="ignore"):
        den, num = raw[D], raw[:D]
        return ~np.isfinite(den) | (den <= 0.0) | ~np.isfinite(num).all(axis=0)


def kernel(ref, target, ref_label, target_label):
    ref = np.asarray(ref, np.float32)
    target = np.asarray(target, np.float32)
    ref_label = np.asarray(ref_label, np.float32)
    labels = np.asarray(target_label).astype(np.int64)

    per_b = _prep_inputs(ref, target, ref_label)
    raws = _run_cores(per_b, SHIFT0)

    # Rescue any columns outside the exp window with shifted reruns (a no-op
    # for data resembling the reference distribution).
    bad = [_bad_cols(r) for r in raws]
    for delta in (60.0, -60.0, 120.0, -120.0):
        if not any(bm.any() for bm in bad):
            break
        raws2 = _run_cores(per_b, SHIFT0 + delta)
        for b in range(B):
            fixable = bad[b] & ~_bad_cols(raws2[b])
            raws[b][:, fixable] = raws2[b][:, fixable]
            bad[b] &= ~fixable

    nll_sum = 0.0
    with np.errstate(all="ignore"):
        for b in range(B):
            pred = raws[b][:D] / raws[b][D]                  # [16, 4096]
            logp = np.log(pred + EPS)
            idx = labels[b].reshape(T)
            nll_sum += -logp[idx, np.arange(T)].sum()
    loss = nll_sum / (B * T)
    return np.asarray(loss, dtype=np.float32)
